# revision 24
# baseline (speedup 1.0000x reference)
"""AWPLoss kernel for Trainium2 (8 NeuronCores, pure data-parallel over batch).

Reference semantics (nn_AWPLoss): sample an alignment a ~ Categorical(log_probs)
per (b, t), clone it (f_prop = identity), and compute
    loss = mean(relu(lambda + log_probs[b,t,a] - log_probs[b,t,a_clone])).
Because the alignment is cloned, original_prob and enhanced_prob are the same
tensor, and the loss reduces to mean(relu(fl(lambda + p) - p)) where p is the
log-prob of the chosen class — the value depends on the sample only through
float32 rounding of (lambda + p) - p, i.e. at the ~1e-5 relative level.

This kernel therefore streams all of log_probs through SBUF (the memory
roofline for this problem), takes the greedy sample p = max_c log_probs[b,t,c]
per row (the mode of the categorical — any choice of sample agrees with the
reference to ~2e-5 relative), computes relu((lambda + p) - p) in float32, and
accumulates. Batch B=64 is sharded 8 ways; per-core partial sums are combined
on the host.

Per-core layout: shard [8, 4096, 128] viewed flat as [32768 rows, 128 classes].
Partition p of SBUF owns rows [p*256, (p+1)*256); each tile moves RT rows per
partition (contiguous RT*512 bytes per partition per DMA).
"""

import numpy as np

B, T, C = 64, 4096, 128
N_CORES = 8
B_PER_CORE = B // N_CORES            # 8
ROWS_PER_CORE = B_PER_CORE * T       # 32768
ROWS_PER_PART = ROWS_PER_CORE // 128  # 256 rows owned by each SBUF partition
RT = 32                              # max rows per partition per tile
# Rows-per-partition per tile: 2 MiB tiles amortize the per-DMA overhead of
# the single SP HWDGE FIFO ring; the tapered tail shrinks the final reduce
# (the only DVE work not hidden behind DMA) from 4.4 us to 1.2 us.
SIZES = [32, 32, 32, 32, 32, 32, 32, 24, 8]
assert sum(SIZES) == ROWS_PER_PART
N_TILES = len(SIZES)
LAMBDA = 0.01
PIPE_DEPTH = 3  # stream DMAs allowed in flight

_NC_CACHE = {}


def _build_bass():
    """Raw Bass (no TileContext): avoids Tile's entry EVSEM barrier and its
    kernel-tail drain + butterfly + sem-reset (~13 us of fixed overhead).

    Two engines: SP issues the 8 stream DMAs (throttled to PIPE_DEPTH in
    flight via dve_sem so tiles land one at a time and DVE overlaps), DVE
    reduces each tile. SP also stores the result. One semaphore per tile:
    HWDGE completions on different lanes are unordered, so a shared
    cumulative semaphore would race.
    """
    from contextlib import ExitStack

    import concourse.bass as bass
    import concourse.mybir as mybir

    nc = bass.Bass()
    x = nc.dram_tensor(
        "x", [ROWS_PER_CORE, C], mybir.dt.float32, kind="ExternalInput"
    )
    partial = nc.dram_tensor(
        "partial", [128, RT], mybir.dt.float32, kind="ExternalOutput"
    )

    # [128, ROWS_PER_PART*C]: partition p's line = rows p*256..(p+1)*256 flat.
    xv = x[:, :].rearrange("(p b) c -> p (b c)", p=128)

    with ExitStack() as ctx:
        # Manual BassBlock so the exit can skip the ~5 us all-engine EVSEM
        # barrier: with only SP and DVE active and the store already waited
        # on, NEFF completion (all queues drained) needs no extra barrier.
        block = bass.BassBlock(nc, "b0")
        block.__enter__()
        dve_sem = ctx.enter_context(nc.semaphore("dve_sem"))
        out_sem = ctx.enter_context(nc.semaphore("out_sem"))
        tile_sems = [
            ctx.enter_context(nc.semaphore(f"ts{t}")) for t in range(N_TILES)
        ]
        tiles = [
            ctx.enter_context(
                nc.sbuf_tensor(f"s{t}", [128, SIZES[t] * C], mybir.dt.float32)
            )
            for t in range(N_TILES)
        ]
        acc = ctx.enter_context(nc.sbuf_tensor("acc", [128, RT], mybir.dt.float32))
        pmax = ctx.enter_context(nc.sbuf_tensor("pmax", [128, RT], mybir.dt.float32))
        dbuf = ctx.enter_context(nc.sbuf_tensor("dbuf", [128, RT], mybir.dt.float32))

        offs = [sum(SIZES[:t]) for t in range(N_TILES)]
        # DVE progress ticks on dve_sem (every DVE op increments it; dependent
        # same-engine ops must wait — the DVE pipeline needs explicit sem sync
        # for RAW/WAR, same as Tile emits):
        #   memset -> 1; tile t: reduce -> 3t+2, stt -> 3t+3, acc -> 3t+4.
        @block.sync
        def _(sync: bass.BassEngine):
            for t in range(N_TILES):
                if t >= PIPE_DEPTH:
                    # reduce of tile t-PIPE_DEPTH done -> its bandwidth
                    # share is clear; keeps PIPE_DEPTH loads in flight.
                    sync.wait_ge(dve_sem, 3 * (t - PIPE_DEPTH) + 2)
                sync.dma_start(
                    out=tiles[t][:, :],
                    in_=xv[:, offs[t] * C : (offs[t] + SIZES[t]) * C],
                ).then_inc(tile_sems[t], 16)
            sync.wait_ge(dve_sem, 3 * N_TILES + 1)
            sync.dma_start(out=partial[:, :], in_=acc[:, :]).then_inc(out_sem, 16)
            sync.wait_ge(out_sem, 16)

        @block.vector
        def _(vector: bass.BassEngine):
            vector.memset(acc[:, :], 0.0).then_inc(dve_sem, 1)
            for t in range(N_TILES):
                rt = SIZES[t]
                vector.wait_ge(tile_sems[t], 16)
                if t > 0:
                    # WAR/WAW on pmax vs previous tile's stt read.
                    vector.wait_ge(dve_sem, 3 * t)
                nc.vector.reduce_max(
                    out=pmax[:, :rt],
                    in_=tiles[t][:, :].rearrange("p (r c) -> p r c", c=C),
                    axis=mybir.AxisListType.X,
                ).then_inc(dve_sem, 1)
                # d = (pmax + LAMBDA) - pmax, in float32, matching the
                # reference's (LAMBDA + p) - p evaluation order.
                vector.wait_ge(dve_sem, 3 * t + 2)
                nc.vector.scalar_tensor_tensor(
                    out=dbuf[:, :rt],
                    in0=pmax[:, :rt],
                    scalar=LAMBDA,
                    in1=pmax[:, :rt],
                    op0=mybir.AluOpType.add,
                    op1=mybir.AluOpType.subtract,
                ).then_inc(dve_sem, 1)
                # acc[:, :rt] += relu(d); host sums every acc slot, so
                # accumulating short tiles into a prefix is fine.
                vector.wait_ge(dve_sem, 3 * t + 3)
                nc.vector.scalar_tensor_tensor(
                    out=acc[:, :rt],
                    in0=dbuf[:, :rt],
                    scalar=0.0,
                    in1=acc[:, :rt],
                    op0=mybir.AluOpType.max,
                    op1=mybir.AluOpType.add,
                ).then_inc(dve_sem, 1)

        # Barrier-free block finalize (BassBlock.__exit__ minus the
        # all_engine_barrier).
        for engine, last_body in block.last_body.items():
            with nc.body(
                last_body, parent=nc.cur_bb, allow_existing_parent=True
            ):
                engine.br(block.end_bb)
        nc.switch_bb(block.end_bb)

    _use_add_imm_sem_updates(nc)
    return nc


def _use_add_imm_sem_updates(nc):
    """then_inc emits update_mode='sem-inc' (event-accelerator path); Tile
    emits 'sem-add-imm', which measures ~0.9 us faster per DVE op on HW.
    Rewrite in place."""
    import concourse.mybir as mybir

    ok = ("InstTensorReduce", "InstTensorScalarPtr", "InstMemSet", "InstDMACopy")
    for f in nc.m.functions:
        for blk in f.blocks:
            for inst in blk.instructions:
                if type(inst).__name__ not in ok:
                    continue
                si = inst.sync_info
                if si and si.on_update:
                    si.on_update = [
                        mybir.SyncUpdate(
                            sync_type=u.sync_type,
                            id=u.id,
                            ant_name=u.ant_name,
                            update_mode="sem-add-imm",
                            update_value=u.update_value,
                            update_reg=u.update_reg,
                        )
                        if u.update_mode == "sem-inc"
                        else u
                        for u in si.on_update
                    ]
                    inst.sync_info = si


def _get_nc():
    if "nc" not in _NC_CACHE:
        _NC_CACHE["nc"] = _build_bass()
    return _NC_CACHE["nc"]


def _run(lp, trace=False):
    from concourse.bass_utils import run_bass_kernel_spmd

    in_maps = [
        {"x": np.ascontiguousarray(lp[c * B_PER_CORE : (c + 1) * B_PER_CORE]).reshape(
            ROWS_PER_CORE, C
        )}
        for c in range(N_CORES)
    ]
    return run_bass_kernel_spmd(
        _get_nc(), in_maps, core_ids=list(range(N_CORES)), trace=trace
    )


def kernel(log_probs, targets=None, input_lengths=None, target_lengths=None):
    lp = np.asarray(log_probs, dtype=np.float32)
    assert lp.shape == (B, T, C), lp.shape
    res = _run(lp)
    total = sum(r["partial"].sum(dtype=np.float64) for r in res.results)
    return np.asarray(total / (B * T), dtype=np.float32)


# revision 25
# speedup vs baseline: 1.1685x; 1.1685x over previous
"""AWPLoss kernel for Trainium2 (8 NeuronCores, pure data-parallel over batch).

Reference semantics (nn_AWPLoss): sample an alignment a ~ Categorical(log_probs)
per (b, t), clone it (f_prop = identity), and compute
    loss = mean(relu(lambda + log_probs[b,t,a] - log_probs[b,t,a_clone])).
Because the alignment is cloned, original_prob and enhanced_prob are the same
tensor, and the loss reduces to mean(relu(fl(lambda + p) - p)) where p is the
log-prob of the chosen class — the value depends on the sample only through
float32 rounding of (lambda + p) - p, i.e. at the ~1e-5 relative level.

This kernel therefore streams all of log_probs through SBUF (the memory
roofline for this problem), takes the greedy sample p = max_c log_probs[b,t,c]
per row (the mode of the categorical — any choice of sample agrees with the
reference to ~2e-5 relative), computes relu((lambda + p) - p) in float32, and
accumulates. Batch B=64 is sharded 8 ways; per-core partial sums are combined
on the host.

Per-core layout: shard [8, 4096, 128] viewed flat as [32768 rows, 128 classes].
Partition p of SBUF owns rows [p*256, (p+1)*256); each tile moves RT rows per
partition (contiguous RT*512 bytes per partition per DMA).
"""

import numpy as np

B, T, C = 64, 4096, 128
N_CORES = 8
B_PER_CORE = B // N_CORES            # 8
ROWS_PER_CORE = B_PER_CORE * T       # 32768
ROWS_PER_PART = ROWS_PER_CORE // 128  # 256 rows owned by each SBUF partition
RT = 32                              # max rows per partition per tile
# Rows-per-partition per tile: 2 MiB tiles amortize the per-DMA overhead of
# the single SP HWDGE FIFO ring; the tapered tail shrinks the final reduce
# (the only DVE work not hidden behind DMA) from 4.4 us to 1.2 us.
SIZES = [32, 32, 32, 32, 32, 32, 32, 24, 8]
assert sum(SIZES) == ROWS_PER_PART
N_TILES = len(SIZES)
LAMBDA = 0.01
PIPE_DEPTH = 3  # stream DMAs allowed in flight

_NC_CACHE = {}


def _build_bass():
    """Raw Bass (no TileContext): avoids Tile's entry EVSEM barrier and its
    kernel-tail drain + butterfly + sem-reset (~13 us of fixed overhead).

    Two engines: SP issues the 8 stream DMAs (throttled to PIPE_DEPTH in
    flight via dve_sem so tiles land one at a time and DVE overlaps), DVE
    reduces each tile. SP also stores the result. One semaphore per tile:
    HWDGE completions on different lanes are unordered, so a shared
    cumulative semaphore would race.
    """
    from contextlib import ExitStack

    import concourse.bass as bass
    import concourse.mybir as mybir

    nc = bass.Bass()
    x = nc.dram_tensor(
        "x", [ROWS_PER_CORE, C], mybir.dt.float32, kind="ExternalInput"
    )
    partial = nc.dram_tensor(
        "partial", [128, RT], mybir.dt.float32, kind="ExternalOutput"
    )

    # [128, ROWS_PER_PART*C]: partition p's line = rows p*256..(p+1)*256 flat.
    xv = x[:, :].rearrange("(p b) c -> p (b c)", p=128)

    with ExitStack() as ctx:
        # Manual BassBlock so the exit can skip the ~5 us all-engine EVSEM
        # barrier: with only SP and DVE active and the store already waited
        # on, NEFF completion (all queues drained) needs no extra barrier.
        block = bass.BassBlock(nc, "b0")
        block.__enter__()
        dve_sem = ctx.enter_context(nc.semaphore("dve_sem"))
        out_sem = ctx.enter_context(nc.semaphore("out_sem"))
        tile_sems = [
            ctx.enter_context(nc.semaphore(f"ts{t}")) for t in range(N_TILES)
        ]
        tiles = [
            ctx.enter_context(
                nc.sbuf_tensor(f"s{t}", [128, SIZES[t] * C], mybir.dt.float32)
            )
            for t in range(N_TILES)
        ]
        acc = ctx.enter_context(nc.sbuf_tensor("acc", [128, RT], mybir.dt.float32))
        pmax = ctx.enter_context(nc.sbuf_tensor("pmax", [128, RT], mybir.dt.float32))
        dbuf = ctx.enter_context(nc.sbuf_tensor("dbuf", [128, RT], mybir.dt.float32))

        offs = [sum(SIZES[:t]) for t in range(N_TILES)]
        # DVE progress ticks on dve_sem (every DVE op increments it; dependent
        # same-engine ops must wait — the DVE pipeline needs explicit sem sync
        # for RAW/WAR, same as Tile emits):
        #   memset -> 1; tile t: reduce -> 3t+2, stt -> 3t+3, acc -> 3t+4.
        # No issue throttle: every tile has its own buffer and the SP HWDGE
        # ring is FIFO, so tiles land strictly in order and back-to-back
        # issue keeps the ring fed — the stream becomes one continuous burst.
        @block.sync
        def _(sync: bass.BassEngine):
            for t in range(N_TILES):
                sync.dma_start(
                    out=tiles[t][:, :],
                    in_=xv[:, offs[t] * C : (offs[t] + SIZES[t]) * C],
                ).then_inc(tile_sems[t], 16)
            sync.wait_ge(dve_sem, 3 * N_TILES + 1)
            sync.dma_start(out=partial[:, :], in_=acc[:, :]).then_inc(out_sem, 16)
            sync.wait_ge(out_sem, 16)

        @block.vector
        def _(vector: bass.BassEngine):
            vector.memset(acc[:, :], 0.0).then_inc(dve_sem, 1)
            for t in range(N_TILES):
                rt = SIZES[t]
                vector.wait_ge(tile_sems[t], 16)
                if t > 0:
                    # WAR/WAW on pmax vs previous tile's stt read.
                    vector.wait_ge(dve_sem, 3 * t)
                nc.vector.reduce_max(
                    out=pmax[:, :rt],
                    in_=tiles[t][:, :].rearrange("p (r c) -> p r c", c=C),
                    axis=mybir.AxisListType.X,
                ).then_inc(dve_sem, 1)
                # d = (pmax + LAMBDA) - pmax, in float32, matching the
                # reference's (LAMBDA + p) - p evaluation order.
                vector.wait_ge(dve_sem, 3 * t + 2)
                nc.vector.scalar_tensor_tensor(
                    out=dbuf[:, :rt],
                    in0=pmax[:, :rt],
                    scalar=LAMBDA,
                    in1=pmax[:, :rt],
                    op0=mybir.AluOpType.add,
                    op1=mybir.AluOpType.subtract,
                ).then_inc(dve_sem, 1)
                # acc[:, :rt] += relu(d); host sums every acc slot, so
                # accumulating short tiles into a prefix is fine.
                vector.wait_ge(dve_sem, 3 * t + 3)
                nc.vector.scalar_tensor_tensor(
                    out=acc[:, :rt],
                    in0=dbuf[:, :rt],
                    scalar=0.0,
                    in1=acc[:, :rt],
                    op0=mybir.AluOpType.max,
                    op1=mybir.AluOpType.add,
                ).then_inc(dve_sem, 1)

        # Barrier-free block finalize (BassBlock.__exit__ minus the
        # all_engine_barrier).
        for engine, last_body in block.last_body.items():
            with nc.body(
                last_body, parent=nc.cur_bb, allow_existing_parent=True
            ):
                engine.br(block.end_bb)
        nc.switch_bb(block.end_bb)

    _use_add_imm_sem_updates(nc)
    return nc


def _use_add_imm_sem_updates(nc):
    """then_inc emits update_mode='sem-inc' (event-accelerator path); Tile
    emits 'sem-add-imm', which measures ~0.9 us faster per DVE op on HW.
    Rewrite in place."""
    import concourse.mybir as mybir

    ok = ("InstTensorReduce", "InstTensorScalarPtr", "InstMemSet", "InstDMACopy")
    for f in nc.m.functions:
        for blk in f.blocks:
            for inst in blk.instructions:
                if type(inst).__name__ not in ok:
                    continue
                si = inst.sync_info
                if si and si.on_update:
                    si.on_update = [
                        mybir.SyncUpdate(
                            sync_type=u.sync_type,
                            id=u.id,
                            ant_name=u.ant_name,
                            update_mode="sem-add-imm",
                            update_value=u.update_value,
                            update_reg=u.update_reg,
                        )
                        if u.update_mode == "sem-inc"
                        else u
                        for u in si.on_update
                    ]
                    inst.sync_info = si


def _get_nc():
    if "nc" not in _NC_CACHE:
        _NC_CACHE["nc"] = _build_bass()
    return _NC_CACHE["nc"]


def _run(lp, trace=False):
    from concourse.bass_utils import run_bass_kernel_spmd

    in_maps = [
        {"x": np.ascontiguousarray(lp[c * B_PER_CORE : (c + 1) * B_PER_CORE]).reshape(
            ROWS_PER_CORE, C
        )}
        for c in range(N_CORES)
    ]
    return run_bass_kernel_spmd(
        _get_nc(), in_maps, core_ids=list(range(N_CORES)), trace=trace
    )


def kernel(log_probs, targets=None, input_lengths=None, target_lengths=None):
    lp = np.asarray(log_probs, dtype=np.float32)
    assert lp.shape == (B, T, C), lp.shape
    res = _run(lp)
    total = sum(r["partial"].sum(dtype=np.float64) for r in res.results)
    return np.asarray(total / (B * T), dtype=np.float32)


# revision 26
# speedup vs baseline: 1.1721x; 1.0031x over previous
"""AWPLoss kernel for Trainium2 (8 NeuronCores, pure data-parallel over batch).

Reference semantics (nn_AWPLoss): sample an alignment a ~ Categorical(log_probs)
per (b, t), clone it (f_prop = identity), and compute
    loss = mean(relu(lambda + log_probs[b,t,a] - log_probs[b,t,a_clone])).
Because the alignment is cloned, original_prob and enhanced_prob are the same
tensor, and the loss reduces to mean(relu(fl(lambda + p) - p)) where p is the
log-prob of the chosen class — the value depends on the sample only through
float32 rounding of (lambda + p) - p, i.e. at the ~1e-5 relative level.

This kernel therefore streams all of log_probs through SBUF (the memory
roofline for this problem), takes the greedy sample p = max_c log_probs[b,t,c]
per row (the mode of the categorical — any choice of sample agrees with the
reference to ~2e-5 relative), computes relu((lambda + p) - p) in float32, and
accumulates. Batch B=64 is sharded 8 ways; per-core partial sums are combined
on the host.

Per-core layout: shard [8, 4096, 128] viewed flat as [32768 rows, 128 classes].
Partition p of SBUF owns rows [p*256, (p+1)*256); each tile moves RT rows per
partition (contiguous RT*512 bytes per partition per DMA).
"""

import numpy as np

B, T, C = 64, 4096, 128
N_CORES = 8
B_PER_CORE = B // N_CORES            # 8
ROWS_PER_CORE = B_PER_CORE * T       # 32768
ROWS_PER_PART = ROWS_PER_CORE // 128  # 256 rows owned by each SBUF partition
RT = 32                              # max rows per partition per tile
# Rows-per-partition per tile: 2 MiB tiles amortize the per-DMA overhead of
# the single SP HWDGE FIFO ring; the tapered tail shrinks the final reduce
# (the only DVE work not hidden behind DMA) from 4.4 us to 1.2 us.
SIZES = [32, 32, 32, 32, 32, 32, 32, 24, 8]
assert sum(SIZES) == ROWS_PER_PART
N_TILES = len(SIZES)
LAMBDA = 0.01
PIPE_DEPTH = 3  # stream DMAs allowed in flight

_NC_CACHE = {}


def _build_bass():
    """Raw Bass (no TileContext): avoids Tile's entry EVSEM barrier and its
    kernel-tail drain + butterfly + sem-reset (~13 us of fixed overhead).

    Two engines: SP issues the 8 stream DMAs (throttled to PIPE_DEPTH in
    flight via dve_sem so tiles land one at a time and DVE overlaps), DVE
    reduces each tile. SP also stores the result. One semaphore per tile:
    HWDGE completions on different lanes are unordered, so a shared
    cumulative semaphore would race.
    """
    from contextlib import ExitStack

    import concourse.bass as bass
    import concourse.mybir as mybir

    nc = bass.Bass()
    x = nc.dram_tensor(
        "x", [ROWS_PER_CORE, C], mybir.dt.float32, kind="ExternalInput"
    )
    partial = nc.dram_tensor(
        "partial", [128, RT], mybir.dt.float32, kind="ExternalOutput"
    )

    # [128, ROWS_PER_PART*C]: partition p's line = rows p*256..(p+1)*256 flat.
    xv = x[:, :].rearrange("(p b) c -> p (b c)", p=128)

    with ExitStack() as ctx:
        # Manual BassBlock so the exit can skip the ~5 us all-engine EVSEM
        # barrier: with only SP and DVE active and the store already waited
        # on, NEFF completion (all queues drained) needs no extra barrier.
        block = bass.BassBlock(nc, "b0")
        block.__enter__()
        dve_sem = ctx.enter_context(nc.semaphore("dve_sem"))
        out_sem = ctx.enter_context(nc.semaphore("out_sem"))
        tile_sems = [
            ctx.enter_context(nc.semaphore(f"ts{t}")) for t in range(N_TILES)
        ]
        tiles = [
            ctx.enter_context(
                nc.sbuf_tensor(f"s{t}", [128, SIZES[t] * C], mybir.dt.float32)
            )
            for t in range(N_TILES)
        ]
        acc = ctx.enter_context(nc.sbuf_tensor("acc", [128, RT], mybir.dt.float32))
        pmax = ctx.enter_context(nc.sbuf_tensor("pmax", [128, RT], mybir.dt.float32))
        dbuf = ctx.enter_context(nc.sbuf_tensor("dbuf", [128, RT], mybir.dt.float32))

        offs = [sum(SIZES[:t]) for t in range(N_TILES)]
        # DVE progress ticks on dve_sem (every DVE op increments it; dependent
        # same-engine ops must wait — the DVE pipeline needs explicit sem sync
        # for RAW/WAR, same as Tile emits):
        #   memset -> 1; tile t: reduce -> 3t+2, stt -> 3t+3, acc -> 3t+4.
        # No issue throttle: every tile has its own buffer and the SP HWDGE
        # ring is FIFO, so tiles land strictly in order and back-to-back
        # issue keeps the ring fed — the stream becomes one continuous burst.
        @block.sync
        def _(sync: bass.BassEngine):
            for t in range(N_TILES):
                sync.dma_start(
                    out=tiles[t][:, :],
                    in_=xv[:, offs[t] * C : (offs[t] + SIZES[t]) * C],
                ).then_inc(tile_sems[t], 16)
            sync.wait_ge(dve_sem, 3 * N_TILES + 1)
            sync.dma_start(out=partial[:, :], in_=acc[:, :]).then_inc(out_sem, 16)
            sync.wait_ge(out_sem, 16)

        @block.vector
        def _(vector: bass.BassEngine):
            vector.memset(acc[:, :], 0.0).then_inc(dve_sem, 1)
            for t in range(N_TILES):
                rt = SIZES[t]
                vector.wait_ge(tile_sems[t], 16)
                if t > 0:
                    # WAR/WAW on pmax vs previous tile's stt read.
                    vector.wait_ge(dve_sem, 3 * t)
                nc.vector.reduce_max(
                    out=pmax[:, :rt],
                    in_=tiles[t][:, :].rearrange("p (r c) -> p r c", c=C),
                    axis=mybir.AxisListType.X,
                ).then_inc(dve_sem, 1)
                # d = (pmax + LAMBDA) - pmax, in float32, matching the
                # reference's (LAMBDA + p) - p evaluation order.
                vector.wait_ge(dve_sem, 3 * t + 2)
                nc.vector.scalar_tensor_tensor(
                    out=dbuf[:, :rt],
                    in0=pmax[:, :rt],
                    scalar=LAMBDA,
                    in1=pmax[:, :rt],
                    op0=mybir.AluOpType.add,
                    op1=mybir.AluOpType.subtract,
                ).then_inc(dve_sem, 1)
                # acc[:, :rt] += relu(d); host sums every acc slot, so
                # accumulating short tiles into a prefix is fine.
                vector.wait_ge(dve_sem, 3 * t + 3)
                nc.vector.scalar_tensor_tensor(
                    out=acc[:, :rt],
                    in0=dbuf[:, :rt],
                    scalar=0.0,
                    in1=acc[:, :rt],
                    op0=mybir.AluOpType.max,
                    op1=mybir.AluOpType.add,
                ).then_inc(dve_sem, 1)

        # Barrier-free block finalize (BassBlock.__exit__ minus the
        # all_engine_barrier).
        for engine, last_body in block.last_body.items():
            with nc.body(
                last_body, parent=nc.cur_bb, allow_existing_parent=True
            ):
                engine.br(block.end_bb)
        nc.switch_bb(block.end_bb)

    _use_add_imm_sem_updates(nc)
    _strip_init_barrier(nc)
    return nc


def _strip_init_barrier(nc):
    """Drop Bass-init const-AP memsets and the init all-engine barrier from
    the 'main' block. Nothing in this kernel reads the const APs, and the
    engines need no common start line — SP can issue the first stream DMA as
    soon as its register preamble is done."""
    for f in nc.m.functions:
        for blk in f.blocks:
            if blk.name != "main":
                continue
            blk.instructions = [
                i
                for i in blk.instructions
                if type(i).__name__
                not in ("InstMemset", "InstDrain", "InstEventSemaphore")
            ]


def _use_add_imm_sem_updates(nc):
    """then_inc emits update_mode='sem-inc' (event-accelerator path); Tile
    emits 'sem-add-imm', which measures ~0.9 us faster per DVE op on HW.
    Rewrite in place."""
    import concourse.mybir as mybir

    ok = ("InstTensorReduce", "InstTensorScalarPtr", "InstMemSet", "InstDMACopy")
    for f in nc.m.functions:
        for blk in f.blocks:
            for inst in blk.instructions:
                if type(inst).__name__ not in ok:
                    continue
                si = inst.sync_info
                if si and si.on_update:
                    si.on_update = [
                        mybir.SyncUpdate(
                            sync_type=u.sync_type,
                            id=u.id,
                            ant_name=u.ant_name,
                            update_mode="sem-add-imm",
                            update_value=u.update_value,
                            update_reg=u.update_reg,
                        )
                        if u.update_mode == "sem-inc"
                        else u
                        for u in si.on_update
                    ]
                    inst.sync_info = si


def _get_nc():
    if "nc" not in _NC_CACHE:
        _NC_CACHE["nc"] = _build_bass()
    return _NC_CACHE["nc"]


def _run(lp, trace=False):
    from concourse.bass_utils import run_bass_kernel_spmd

    in_maps = [
        {"x": np.ascontiguousarray(lp[c * B_PER_CORE : (c + 1) * B_PER_CORE]).reshape(
            ROWS_PER_CORE, C
        )}
        for c in range(N_CORES)
    ]
    return run_bass_kernel_spmd(
        _get_nc(), in_maps, core_ids=list(range(N_CORES)), trace=trace
    )


def kernel(log_probs, targets=None, input_lengths=None, target_lengths=None):
    lp = np.asarray(log_probs, dtype=np.float32)
    assert lp.shape == (B, T, C), lp.shape
    res = _run(lp)
    total = sum(r["partial"].sum(dtype=np.float64) for r in res.results)
    return np.asarray(total / (B * T), dtype=np.float32)
